# revision 1
# baseline (speedup 1.0000x reference)
"""Trainium2 Bass kernel for the supervised-contrastive loss (nn_KCL_69784628626020).

Strategy (8 NeuronCores, SPMD):
  - Shard anchors (rows of q, k, y) across cores: 1024 rows/core.
  - Each core computes its [1024, 8192] slab of the score matrix
    S = q_loc @ q_full^T on the tensor engine (float32r, full rate at N>=512).
  - The per-column weight w_j = 1/count(y_j) is folded into the matmul as an
    extra K=1 rank-1 update adding TAU*ln(w_j) to the scores, so that the
    scalar engine's exp(PSUM/TAU) directly produces EW_ij = exp(S_ij/TAU)*w_j.
  - Per row i:
        A_i = sum_j  EW_ij            (diag excluded)
        B_i = sum_{y_j==y_i} EW_ij    (diag excluded)
        den_i = log(A_i - B_i)
        num_i = log(kpos_i + c_i * B_i)      # c_i = count(y_i), B*c = unweighted
        loss_i = (den_i - num_i) / (c_i - 1 + K)
    A and B each come from ONE fused DVE scalar_tensor_tensor op per tile
    (compare + multiply + row-reduce).  Diagonal exclusion is data-driven
    (host-provided global row ids compared against a column iota), so the
    program is identical across cores (SPMD-safe).
  - Class counts are computed on device: row-sums of the y-equality mask give
    count(y_i) for local rows; an AllGather assembles counts for all 8192
    columns.
  - kpos_i = sum_k exp(q_i . k_ik / TAU) via fused multiply-reduce per k.
  - Final mean: per-core partial sum via a ones-matmul partition reduction;
    host adds the 8 partials (the unshard step).
"""

import numpy as np
from contextlib import ExitStack

import concourse.bass as bass
import concourse.bacc as bacc
import concourse.tile as tile
from concourse import mybir
from concourse.bass_utils import run_bass_kernel_spmd
import ml_dtypes

F32 = mybir.dt.float32
F32R = mybir.dt.float32r
F16 = mybir.dt.float16
BF16 = mybir.dt.bfloat16

TAU = 0.07
NCORES = 8


class Cfg:
    def __init__(self, N=8192, D=512, KP=8, TW=1024):
        self.N = N            # total rows (anchors)
        self.D = D            # feature dim
        self.KP = KP          # external positives per anchor
        self.TW = TW          # column tile width
        self.NL = N // NCORES     # rows per core
        self.NB = self.NL // 128  # row blocks per core
        self.NS = N // TW         # column tiles
        self.KC = D // 128        # contraction chunks
        assert self.NL % 128 == 0 and N % TW == 0 and D % 128 == 0
        assert TW % 512 == 0
        self.NCH = TW // 512      # 512-wide matmul chunks per column tile


# Engine selection knobs (tuned from traces).
STT1_ENGINES = None  # set in build_bass
STT2_ENGINES = None
KPATH_ENGINES = None


def build_bass(cfg: Cfg, stt1_eng="vector", stt2_eng="vector", k_eng="vector"):
    N, D, KP, TW = cfg.N, cfg.D, cfg.KP, cfg.TW
    NL, NB, NS, KC, NCH = cfg.NL, cfg.NB, cfg.NS, cfg.KC, cfg.NCH

    nc = bacc.Bacc("TRN2", target_bir_lowering=False, debug=False,
                   num_devices=NCORES)

    # ---- kernel I/O -------------------------------------------------------
    qT_d = nc.dram_tensor("qT", [KC, 128, N], F32R, kind="ExternalInput")
    qTl_d = nc.dram_tensor("qTl", [KC, 128, NL], F32R, kind="ExternalInput")
    kr_d = nc.dram_tensor("kr", [NB, 128, KP * D], BF16, kind="ExternalInput")
    qr_d = nc.dram_tensor("qr", [NB, 128, D], F32, kind="ExternalInput")
    ybc_d = nc.dram_tensor("ybc", [128, N], F16, kind="ExternalInput")
    yrow_d = nc.dram_tensor("yrow", [128, NB], F32, kind="ExternalInput")
    colid_d = nc.dram_tensor("colid", [128, TW], F16, kind="ExternalInput")
    rowadj_d = nc.dram_tensor("rowadj", [128, NB * NS], F32, kind="ExternalInput")
    out_d = nc.dram_tensor("out", [1, 1], F32, kind="ExternalOutput")

    eng = {"vector": nc.vector, "gpsimd": nc.gpsimd}
    stt1e = eng[stt1_eng]
    stt2e = eng[stt2_eng]
    ke = eng[k_eng]

    with tile.TileContext(nc) as tc, ExitStack() as ctx:
        const = ctx.enter_context(tc.tile_pool(name="const", bufs=1))
        rh_pool = ctx.enter_context(tc.tile_pool(name="rh", bufs=8))
        psum_pool = ctx.enter_context(tc.tile_pool(name="ps", bufs=3, space="PSUM"))
        ew_pool = ctx.enter_context(tc.tile_pool(name="ew", bufs=3))
        t1_pool = ctx.enter_context(tc.tile_pool(name="t1", bufs=3))
        t2_pool = ctx.enter_context(tc.tile_pool(name="t2", bufs=2))
        k_pool = ctx.enter_context(tc.tile_pool(name="kp", bufs=2))
        q_pool = ctx.enter_context(tc.tile_pool(name="qp", bufs=2))
        dram = ctx.enter_context(tc.tile_pool(name="dram", bufs=1, space="DRAM"))

        # ---- resident constants ------------------------------------------
        qtl = [const.tile([128, NL], F32R, tag=f"qtl{c}", name=f"qtl{c}") for c in range(KC)]
        for c in range(KC):
            nc.sync.dma_start(qtl[c][:, :], qTl_d[c, :, :])
        ybc = const.tile([128, N], F16, tag="ybc")
        nc.sync.dma_start(ybc[:, :], ybc_d[:, :])
        colid = const.tile([128, TW], F16, tag="colid")
        nc.sync.dma_start(colid[:, :], colid_d[:, :])
        yrow = const.tile([128, NB], F32, tag="yrow")
        nc.sync.dma_start(yrow[:, :], yrow_d[:, :])
        rowadj = const.tile([128, NB * NS], F32, tag="rowadj")
        nc.sync.dma_start(rowadj[:, :], rowadj_d[:, :])

        ones_k1 = const.tile([1, 128], F32R, tag="ones_k1")
        nc.vector.memset(ones_k1[:, :].bitcast(F32), 1.0)
        ones_col = const.tile([128, 1], F32, tag="ones_col")
        nc.vector.memset(ones_col[:, :], 1.0)

        # accumulator slots
        aslt = const.tile([128, NB * NS], F32, tag="aslt")
        bslt = const.tile([128, NB * NS], F32, tag="bslt")
        kss = const.tile([128, NB * KP], F32, tag="kss")
        kpos = const.tile([128, NB], F32, tag="kpos")
        cloc = const.tile([128, NB], F32, tag="cloc")
        losscol = const.tile([128, NB], F32, tag="losscol")

        # ---- phase W: class counts + lw ----------------------------------
        cnt_scr = const.tile([128, N], F16, tag="cnt_scr")
        for b in range(NB):
            nc.vector.tensor_scalar(
                cnt_scr[:, :], ybc[:, :], yrow[:, b:b + 1], None,
                op0=mybir.AluOpType.is_equal,
                op1=mybir.AluOpType.add,
                accum_out=cloc[:, b:b + 1])

        cpart = dram.tile([1, NL], F32)
        call = dram.tile([NCORES, NL], F32, addr_space="Shared")
        # cpart[0, b*128+p] = cloc[p, b]
        nc.sync.dma_start(
            cpart[:, :].rearrange("o (b p) -> p (o b)", b=NB, p=128),
            cloc[:, :])
        nc.gpsimd.collective_compute(
            "AllGather", mybir.AluOpType.bypass,
            ins=[cpart[:, :].opt()],
            outs=[call[:, :].opt()],
            replica_groups=[list(range(NCORES))],
        )
        # counts for all N columns -> SBUF [128, N/128] (global row-major)
        NF = N // 128
        csb = const.tile([128, NF], F32, tag="csb")
        nc.sync.dma_start(
            csb[:, :],
            call[:, :].rearrange("r l -> (r l)").rearrange("(p f) -> p f", p=128, f=NF))
        lnc = const.tile([128, NF], F32, tag="lnc")
        nc.scalar.activation(lnc[:, :], csb[:, :], mybir.ActivationFunctionType.Ln)
        lwsb = const.tile([128, NF], F32R, tag="lwsb")
        nc.vector.tensor_scalar_mul(lwsb[:, :], lnc[:, :], -TAU)
        lw_d = dram.tile([1, N], F32R)
        nc.sync.dma_start(
            lw_d[:, :].rearrange("o (p f) -> p (o f)", p=128, f=NF),
            lwsb[:, :])
        lwrow = const.tile([1, N], F32R, tag="lwrow")
        nc.sync.dma_start(lwrow[:, :], lw_d[:, :])

        # ---- main loop: score slab ---------------------------------------
        for s in range(NS):
            rhs = [rh_pool.tile([128, TW], F32R, tag="rh", name=f"rhs{s}_{c2}") for c2 in range(KC)]
            for c in range(KC):
                nc.sync.dma_start(rhs[c][:, :], qT_d[c, :, s * TW:(s + 1) * TW])
            for b in range(NB):
                ps = psum_pool.tile([128, TW], F32)
                for nch in range(NCH):
                    o = ps[:, nch * 512:(nch + 1) * 512]
                    for c in range(KC):
                        nc.tensor.matmul(
                            o,
                            qtl[c][:, b * 128:(b + 1) * 128],
                            rhs[c][:, nch * 512:(nch + 1) * 512],
                            start=(c == 0), stop=False)
                    nc.tensor.matmul(
                        o,
                        ones_k1[0:1, :],
                        lwrow[0:1, s * TW + nch * 512: s * TW + (nch + 1) * 512],
                        start=False, stop=True)
                ew = ew_pool.tile([128, TW], F32)
                nc.scalar.activation(ew[:, :], ps[:, :],
                                     mybir.ActivationFunctionType.Exp,
                                     scale=float(1.0 / TAU))
                # A: zero the diagonal, row-sum everything
                t1 = t1_pool.tile([128, TW], F32)
                stt1e.scalar_tensor_tensor(
                    t1[:, :], colid[:, :], rowadj[:, (b * NS + s):(b * NS + s) + 1],
                    ew[:, :],
                    op0=mybir.AluOpType.not_equal, op1=mybir.AluOpType.mult,
                    accum_out=aslt[:, (b * NS + s):(b * NS + s) + 1])
                # B: same-class row-sum (diag already zeroed in t1)
                t2 = t2_pool.tile([128, TW], F16)
                stt2e.scalar_tensor_tensor(
                    t2[:, :], ybc[:, s * TW:(s + 1) * TW], yrow[:, b:b + 1],
                    t1[:, :],
                    op0=mybir.AluOpType.is_equal, op1=mybir.AluOpType.mult,
                    accum_out=bslt[:, (b * NS + s):(b * NS + s) + 1])

        # ---- k-path: kpos = sum_k exp(q.k/TAU) ---------------------------
        for b in range(NB):
            kt = k_pool.tile([128, KP * D], BF16, tag="kt")
            nc.sync.dma_start(kt[:, :], kr_d[b, :, :])
            qt = q_pool.tile([128, D], F32, tag="qt")
            nc.sync.dma_start(qt[:, :], qr_d[b, :, :])
            for kk in range(KP):
                kscr = q_pool.tile([128, D], BF16, tag="kscr")
                ke.scalar_tensor_tensor(
                    kscr[:, :], kt[:, kk * D:(kk + 1) * D], 1.0,
                    qt[:, :],
                    op0=mybir.AluOpType.mult, op1=mybir.AluOpType.mult,
                    accum_out=kss[:, b * KP + kk: b * KP + kk + 1])
            ksse = const.tile([128, KP], F32, tag=f"ksse{b}")
            nc.scalar.activation(
                ksse[:, :],
                kss[:, b * KP:(b + 1) * KP],
                mybir.ActivationFunctionType.Exp, scale=float(1.0 / TAU),
                accum_out=kpos[:, b:b + 1])

        # ---- finalize per row block --------------------------------------
        fin = const.tile([128, 6 * NB], F32, tag="fin")
        for b in range(NB):
            acol = fin[:, 6 * b + 0: 6 * b + 1]
            bcol = fin[:, 6 * b + 1: 6 * b + 2]
            nc.vector.tensor_reduce(acol, aslt[:, b * NS:(b + 1) * NS],
                                    mybir.AxisListType.X, mybir.AluOpType.add)
            nc.vector.tensor_reduce(bcol, bslt[:, b * NS:(b + 1) * NS],
                                    mybir.AxisListType.X, mybir.AluOpType.add)
            den_in = fin[:, 6 * b + 2: 6 * b + 3]
            nc.vector.tensor_sub(den_in, acol, bcol)
            num_in = fin[:, 6 * b + 3: 6 * b + 4]
            # num_in = kpos + cloc * B
            nc.vector.scalar_tensor_tensor(
                num_in, bcol, cloc[:, b:b + 1], kpos[:, b:b + 1],
                op0=mybir.AluOpType.mult, op1=mybir.AluOpType.add)
            den_l = fin[:, 6 * b + 4: 6 * b + 5]
            nc.scalar.activation(den_l, den_in, mybir.ActivationFunctionType.Ln)
            num_l = fin[:, 6 * b + 5: 6 * b + 6]
            nc.scalar.activation(num_l, num_in, mybir.ActivationFunctionType.Ln)
        # losscol[:, b] = (den_l - num_l) / (cloc - 1 + KP)
        dinv_t = const.tile([128, NB], F32, tag="dinv")
        tmp_t = const.tile([128, NB], F32, tag="tmpd")
        nc.vector.tensor_scalar_add(tmp_t[:, :], cloc[:, :], float(KP - 1))
        nc.vector.reciprocal(dinv_t[:, :], tmp_t[:, :])
        for b in range(NB):
            den_l = fin[:, 6 * b + 4: 6 * b + 5]
            num_l = fin[:, 6 * b + 5: 6 * b + 6]
            diff = fin[:, 6 * b + 2: 6 * b + 3]  # overwrite den_in
            nc.vector.tensor_sub(diff, den_l, num_l)
            nc.vector.tensor_mul(losscol[:, b:b + 1], diff, dinv_t[:, b:b + 1])

        # ---- reduce to a single partial ----------------------------------
        lsum = const.tile([128, 1], F32, tag="lsum")
        nc.vector.tensor_reduce(lsum[:, :], losscol[:, :],
                                mybir.AxisListType.X, mybir.AluOpType.add)
        psf = psum_pool.tile([128, 512], F32, bufs=1)
        nc.tensor.matmul(psf[0:1, 0:1], lsum[:, :],
                         ones_col[:, :], start=True, stop=True)
        outsb = const.tile([1, 1], F32, tag="outsb")
        nc.scalar.copy(outsb[0:1, 0:1], psf[0:1, 0:1])
        nc.sync.dma_start(out_d[:, :], outsb[0:1, 0:1])

    nc.compile()
    return nc


# ---------------------------------------------------------------------------
# host-side marshalling
# ---------------------------------------------------------------------------

def make_inputs(q, k, y, cfg: Cfg):
    """Build the per-core input maps (pure layout/replication marshalling)."""
    N, D, KP, TW = cfg.N, cfg.D, cfg.KP, cfg.TW
    NL, NB, NS, KC = cfg.NL, cfg.NB, cfg.NS, cfg.KC
    q = np.asarray(q, dtype=np.float32)
    k = np.asarray(k, dtype=np.float32)
    y = np.asarray(y)

    qT = np.ascontiguousarray(q.T).reshape(KC, 128, N)
    ybc = np.broadcast_to(y.astype(np.float16)[None, :], (128, N)).copy()
    colid = np.broadcast_to(np.arange(TW, dtype=np.float16)[None, :], (128, TW)).copy()

    in_maps = []
    for r in range(NCORES):
        rows = slice(r * NL, (r + 1) * NL)
        qTl = np.ascontiguousarray(q[rows].T).reshape(KC, 128, NL)
        kr = np.ascontiguousarray(k[rows].reshape(NB, 128, KP * D)).astype(ml_dtypes.bfloat16)
        qr = np.ascontiguousarray(q[rows].reshape(NB, 128, D))
        yrow = np.ascontiguousarray(y[rows].astype(np.float32).reshape(NB, 128).T)
        # rowadj[p, b*NS+s] = global_row - s*TW
        p = np.arange(128, dtype=np.float32)
        badx = np.arange(NB, dtype=np.float32)
        sadx = np.arange(NS, dtype=np.float32)
        grow = r * NL + badx[:, None, None] * 128 + p[None, :, None]  # [NB,128,1]
        rowadj = (grow - sadx[None, None, :] * TW)                   # [NB,128,NS]
        rowadj = np.ascontiguousarray(rowadj.transpose(1, 0, 2).reshape(128, NB * NS),
                                      dtype=np.float32)
        in_maps.append({
            "qT": qT, "qTl": qTl, "kr": kr, "qr": qr,
            "ybc": ybc, "yrow": yrow, "colid": colid, "rowadj": rowadj,
        })
    return in_maps


_CACHE = {}


def _get_nc(cfg_key):
    if cfg_key not in _CACHE:
        cfg = Cfg()
        _CACHE[cfg_key] = (cfg, build_bass(cfg))
    return _CACHE[cfg_key]


def kernel(q, k, y, trace=False):
    cfg, nc = _get_nc("full")
    in_maps = make_inputs(q, k, y, cfg)
    res = run_bass_kernel_spmd(nc, in_maps, core_ids=list(range(NCORES)),
                               trace=trace)
    total = np.sum([res.results[r]["out"][0, 0] for r in range(NCORES)],
                   dtype=np.float64)
    out = np.asarray(total / cfg.N, dtype=np.float32)
    if trace:
        kernel.last_results = res
    return out



# revision 14
# speedup vs baseline: 1.6952x; 1.6952x over previous
"""Trainium2 Bass kernel for the supervised-contrastive loss (nn_KCL_69784628626020).

Strategy (8 NeuronCores, SPMD), v2:
  - Shard anchors (rows of q, k, y) across cores: 1024 rows/core.
  - Each core computes its [1024, 8192] slab of S = q_loc @ q_full^T in bf16
    on the tensor engine.  The PE queue has NO dependency on the class-count
    path, so it streams 512 matmuls back-to-back and ramps to the 2.4 GHz
    pstate (vs 1.2 GHz when stalled).
  - Scalar engine: ewu = exp(PSUM/TAU) -> bf16 (UNWEIGHTED exponentials).
  - Diagonal exclusion: only the tile column-strip that can contain the
    diagonal is masked (a [128,128] fused op instead of [128,1024]), using a
    host-provided per-(b,s) strip selector so the program is SPMD-identical.
  - Per row i (all sums exclude the diagonal):
        P_i = sum_{y_j==y_i} ewu_ij          (unweighted same-class)   [DVE]
        T_i = sum_j          ewu_ij * w_j    (weighted total, w=1/c_j) [DVE]
        den_i = log(T_i - P_i / c_i)
        num_i = log(kpos_i + P_i)
        loss_i = (den_i - num_i) / (c_i - 1 + K)
    P and T each come from one fused scalar_tensor_tensor op per tile with
    all-bf16 operands (eligible for the DVE 2x 16-bit mode).
  - w_j needs global class counts: each core counts its own 1024 rows
    (split across DVE and gpsimd), AllGather assembles all 8192, reciprocal
    + a broadcast DMA produce the resident [128, N] bf16 weight table.  The
    T-accumulation is emitted with a LAG of ~24 tiles so the DVE queue never
    stalls waiting for the weight table.
  - kpos_i = sum_k exp(q_i . k_ik / TAU): computed on the TENSOR engine as a
    full [128, 128*K] block product q_b @ k_b^T (one extra matmul tile per
    row block), exp on the scalar engine, then one fused mask-and-row-reduce
    on DVE extracts the generalized diagonal (mask[p,c] = c//K == p).
  - Final mean: per-core partial via a ones-matmul partition reduction;
    host adds the 8 partials (the unshard step).
"""

import numpy as np
from contextlib import ExitStack

import concourse.bass as bass
import concourse.bacc as bacc
import concourse.tile as tile
from concourse import mybir
from concourse.bass_utils import run_bass_kernel_spmd
import ml_dtypes

F32 = mybir.dt.float32
F16 = mybir.dt.float16
BF16 = mybir.dt.bfloat16

TAU = 0.07
NCORES = 8


class Cfg:
    def __init__(self, N=8192, D=512, KP=8, TW=1024):
        self.N = N            # total rows (anchors)
        self.D = D            # feature dim
        self.KP = KP          # external positives per anchor
        self.TW = TW          # column tile width
        self.NL = N // NCORES     # rows per core
        self.NB = self.NL // 128  # row blocks per core
        self.NS = N // TW         # column tiles
        self.KC = D // 128        # contraction chunks
        assert self.NL % 128 == 0 and N % TW == 0 and D % 128 == 0
        assert TW % 512 == 0
        self.NCH = TW // 512      # 512-wide matmul chunks per column tile
        # strip width for diagonal masking: when NL == TW the diagonal of
        # the (unique) diagonal column-tile falls entirely inside the
        # [b*128, b*128+128) strip; otherwise mask the whole tile.
        self.SW = 128 if self.NL == TW else TW


def build_bass(cfg: Cfg, lag=24, trecip=12):
    N, D, KP, TW = cfg.N, cfg.D, cfg.KP, cfg.TW
    NL, NB, NS, KC, NCH, SW = cfg.NL, cfg.NB, cfg.NS, cfg.KC, cfg.NCH, cfg.SW
    TT = NB * NS                       # total tiles
    L = max(1, min(lag, TT - 1))       # tW emission lag
    TR = min(trecip, L - 1) if L > 1 else 0
    KW = KP * 128                      # k-path block product width
    NCHK = KW // 512 if KW % 512 == 0 else 0
    assert NCHK > 0, "KP*128 must be a multiple of 512"

    nc = bacc.Bacc("TRN2", target_bir_lowering=False, debug=False,
                   num_devices=NCORES)

    # ---- kernel I/O -------------------------------------------------------
    qT_d = nc.dram_tensor("qT", [KC, 128, N], BF16, kind="ExternalInput")
    qTl_d = nc.dram_tensor("qTl", [KC, 128, NL], BF16, kind="ExternalInput")
    kTl_d = nc.dram_tensor("kTl", [KC, 128, NL * KP], BF16, kind="ExternalInput")
    ybc_d = nc.dram_tensor("ybc", [128, N], F16, kind="ExternalInput")
    yrow_d = nc.dram_tensor("yrow", [128, NB], F32, kind="ExternalInput")
    colid_d = nc.dram_tensor("colid", [128, SW], F16, kind="ExternalInput")
    strip_d = nc.dram_tensor("strip", [128, NB * NS], F32, kind="ExternalInput")
    kmask_d = nc.dram_tensor("kmask", [128, KW], F16, kind="ExternalInput")
    out_d = nc.dram_tensor("out", [1, 1], F32, kind="ExternalOutput")

    with tile.TileContext(nc) as tc, ExitStack() as ctx:
        const = ctx.enter_context(tc.tile_pool(name="const", bufs=1))
        rh_pool = ctx.enter_context(tc.tile_pool(name="rh", bufs=2 * KC))
        psum_pool = ctx.enter_context(tc.tile_pool(name="ps", bufs=2, space="PSUM"))
        psk_pool = ctx.enter_context(tc.tile_pool(name="psk", bufs=1, space="PSUM"))
        ew_pool = ctx.enter_context(tc.tile_pool(name="ew", bufs=L + 2))
        t2s_pool = ctx.enter_context(tc.tile_pool(name="t2s", bufs=2))
        tws_pool = ctx.enter_context(tc.tile_pool(name="tws", bufs=2))
        k_pool = ctx.enter_context(tc.tile_pool(name="kp", bufs=2 * KC))
        ewk_pool = ctx.enter_context(tc.tile_pool(name="ewk", bufs=2))
        ks_pool = ctx.enter_context(tc.tile_pool(name="ks", bufs=2))
        dram = ctx.enter_context(tc.tile_pool(name="dram", bufs=1, space="DRAM"))

        # ---- resident constants ------------------------------------------
        ybc = const.tile([128, N], F16, tag="ybc")
        nc.sync.dma_start(ybc[:, :], ybc_d[:, :])
        yrow = const.tile([128, NB], F32, tag="yrow")
        nc.sync.dma_start(yrow[:, :], yrow_d[:, :])
        colid = const.tile([128, SW], F16, tag="colid")
        nc.sync.dma_start(colid[:, :], colid_d[:, :])
        strip = const.tile([128, NB * NS], F32, tag="strip")
        nc.sync.dma_start(strip[:, :], strip_d[:, :])
        qtl = [const.tile([128, NL], BF16, tag=f"qtl{c}", name=f"qtl{c}")
               for c in range(KC)]
        for c in range(KC):
            nc.sync.dma_start(qtl[c][:, :], qTl_d[c, :, :])

        ones_col = const.tile([128, 1], F32, tag="ones_col")
        nc.vector.memset(ones_col[:, :], 1.0)

        kmask = const.tile([128, KW], F16, tag="kmask")
        nc.sync.dma_start(kmask[:, :], kmask_d[:, :])

        # accumulator slots
        aslt = const.tile([128, NB * NS], F32, tag="aslt")   # T (weighted)
        bslt = const.tile([128, NB * NS], F32, tag="bslt")   # P (same-class)
        kpos = const.tile([128, NB], F32, tag="kpos")
        cloc = const.tile([128, NB], F32, tag="cloc")
        wbc = const.tile([128, N], BF16, tag="wbc")          # 1/c_j broadcast

        # ---- phase W: local class counts (DVE) ---------------------------
        cntA = const.tile([128, N], F16, tag="cntA")
        for b in range(NB):
            nc.vector.tensor_scalar(
                cntA[:, :], ybc[:, :], yrow[:, b:b + 1], None,
                op0=mybir.AluOpType.is_equal,
                op1=mybir.AluOpType.add,
                accum_out=cloc[:, b:b + 1])

        cpart = dram.tile([1, NL], F32)
        call = dram.tile([NCORES, NL], F32, addr_space="Shared")
        nc.sync.dma_start(
            cpart[:, :].rearrange("o (b p) -> p (o b)", b=NB, p=128),
            cloc[:, :])
        nc.gpsimd.collective_compute(
            "AllGather", mybir.AluOpType.bypass,
            ins=[cpart[:, :].opt()],
            outs=[call[:, :].opt()],
            replica_groups=[list(range(NCORES))],
        )
        NF = N // 128
        csb = const.tile([128, NF], F32, tag="csb")
        nc.sync.dma_start(
            csb[:, :],
            call[:, :].rearrange("r l -> (r l)").rearrange("(p f) -> p f", p=128, f=NF))
        wsb = const.tile([128, NF], BF16, tag="wsb")
        winvf = const.tile([128, NF], F32, tag="winvf")
        wrow_d = dram.tile([1, N], BF16)

        # ---- main loop: score slab ---------------------------------------
        tiles = []     # (ewu tile, b, s) pending tW emission
        ntw = 0        # tWs emitted

        def emit_tw(j):
            ewu_j, b_j, s_j = tiles[j]
            tws = tws_pool.tile([128, TW], BF16, tag="tws")
            nc.vector.scalar_tensor_tensor(
                tws[:, :], wbc[:, s_j * TW:(s_j + 1) * TW], 1.0,
                ewu_j[:, :],
                op0=mybir.AluOpType.mult, op1=mybir.AluOpType.mult,
                accum_out=aslt[:, (b_j * NS + s_j):(b_j * NS + s_j) + 1])

        # k-path blocks are interleaved into the main loop: block b runs
        # after column-tile iteration s == b * NS // NB.
        kpath_after_s = {(b * NS) // NB: b for b in range(NB)}
        assert len(kpath_after_s) == NB, "need NS >= NB for k-path interleave"

        def emit_kpath(b):
            ktl = [k_pool.tile([128, KW], BF16, tag="ktl", name=f"ktl{b}_{c2}")
                   for c2 in range(KC)]
            for c in range(KC):
                nc.sync.dma_start(ktl[c][:, :], kTl_d[c, :, b * KW:(b + 1) * KW])
            psk = psk_pool.tile([128, KW], F32)
            for nch in range(NCHK):
                o = psk[:, nch * 512:(nch + 1) * 512]
                for c in range(KC):
                    nc.tensor.matmul(
                        o,
                        qtl[c][:, b * 128:(b + 1) * 128],
                        ktl[c][:, nch * 512:(nch + 1) * 512],
                        start=(c == 0), stop=(c == KC - 1))
            ewk = ewk_pool.tile([128, KW], BF16, tag="ewk")
            nc.scalar.activation(ewk[:, :], psk[:, :],
                                 mybir.ActivationFunctionType.Exp,
                                 scale=float(1.0 / TAU))
            kscr = ks_pool.tile([128, KW], BF16, tag="kscr")
            nc.vector.scalar_tensor_tensor(
                kscr[:, :], kmask[:, :], 1.0, ewk[:, :],
                op0=mybir.AluOpType.mult, op1=mybir.AluOpType.mult,
                accum_out=kpos[:, b:b + 1])

        for s in range(NS):
            rhs = [rh_pool.tile([128, TW], BF16, tag="rh", name=f"rhs{s}_{c2}")
                   for c2 in range(KC)]
            for c in range(KC):
                nc.sync.dma_start(rhs[c][:, :], qT_d[c, :, s * TW:(s + 1) * TW])
            for b in range(NB):
                k = s * NB + b
                idx = b * NS + s
                ps = psum_pool.tile([128, TW], F32)
                for nch in range(NCH):
                    o = ps[:, nch * 512:(nch + 1) * 512]
                    for c in range(KC):
                        nc.tensor.matmul(
                            o,
                            qtl[c][:, b * 128:(b + 1) * 128],
                            rhs[c][:, nch * 512:(nch + 1) * 512],
                            start=(c == 0), stop=(c == KC - 1))
                ewu = ew_pool.tile([128, TW], BF16)
                nc.scalar.activation(ewu[:, :], ps[:, :],
                                     mybir.ActivationFunctionType.Exp,
                                     scale=float(1.0 / TAU))
                # diagonal strip mask (no-op rows have strip == -1)
                coff = b * 128 if SW == 128 else 0
                nc.vector.scalar_tensor_tensor(
                    ewu[:, coff:coff + SW], colid[:, :],
                    strip[:, idx:idx + 1],
                    ewu[:, coff:coff + SW],
                    op0=mybir.AluOpType.not_equal, op1=mybir.AluOpType.mult)
                # P: same-class row-sum (unweighted)
                t2s = t2s_pool.tile([128, TW], BF16, tag="t2s")
                nc.vector.scalar_tensor_tensor(
                    t2s[:, :], ybc[:, s * TW:(s + 1) * TW], yrow[:, b:b + 1],
                    ewu[:, :],
                    op0=mybir.AluOpType.is_equal, op1=mybir.AluOpType.mult,
                    accum_out=bslt[:, idx:idx + 1])
                tiles.append((ewu, b, s))
                if k == TR:
                    # build the weight table once counts have gathered
                    nc.vector.reciprocal(winvf[:, :], csb[:, :])
                    nc.vector.tensor_scalar_mul(wsb[:, :], winvf[:, :], 1.0)
                    nc.sync.dma_start(
                        wrow_d[:, :].rearrange("o (p f) -> p (o f)", p=128, f=NF),
                        wsb[:, :])
                    nc.sync.dma_start(wbc[:, :],
                                      wrow_d[:, :].broadcast_to([128, N]))
                if k >= L:
                    emit_tw(ntw)
                    ntw += 1
            if s in kpath_after_s:
                emit_kpath(kpath_after_s[s])
        while ntw < TT:
            emit_tw(ntw)
            ntw += 1

        # ---- finalize ----------------------------------------------------
        Tcol = const.tile([128, NB], F32, tag="Tcol")
        Pcol = const.tile([128, NB], F32, tag="Pcol")
        for b in range(NB):
            nc.vector.tensor_reduce(Tcol[:, b:b + 1], aslt[:, b * NS:(b + 1) * NS],
                                    mybir.AxisListType.X, mybir.AluOpType.add)
            nc.vector.tensor_reduce(Pcol[:, b:b + 1], bslt[:, b * NS:(b + 1) * NS],
                                    mybir.AxisListType.X, mybir.AluOpType.add)
        winv = const.tile([128, NB], F32, tag="winv")
        nc.vector.reciprocal(winv[:, :], cloc[:, :])
        tmp = const.tile([128, NB], F32, tag="tmp")
        nc.vector.tensor_tensor(tmp[:, :], Pcol[:, :], winv[:, :],
                                op=mybir.AluOpType.mult)
        den_in = const.tile([128, NB], F32, tag="den_in")
        nc.vector.tensor_tensor(den_in[:, :], Tcol[:, :], tmp[:, :],
                                op=mybir.AluOpType.subtract)
        num_in = const.tile([128, NB], F32, tag="num_in")
        nc.vector.tensor_tensor(num_in[:, :], Pcol[:, :], kpos[:, :],
                                op=mybir.AluOpType.add)
        den_l = const.tile([128, NB], F32, tag="den_l")
        nc.scalar.activation(den_l[:, :], den_in[:, :],
                             mybir.ActivationFunctionType.Ln)
        num_l = const.tile([128, NB], F32, tag="num_l")
        nc.scalar.activation(num_l[:, :], num_in[:, :],
                             mybir.ActivationFunctionType.Ln)
        diff = const.tile([128, NB], F32, tag="diff")
        nc.vector.tensor_tensor(diff[:, :], den_l[:, :], num_l[:, :],
                                op=mybir.AluOpType.subtract)
        dnm = const.tile([128, NB], F32, tag="dnm")
        nc.vector.tensor_scalar_add(dnm[:, :], cloc[:, :], float(KP - 1))
        dinv = const.tile([128, NB], F32, tag="dinv")
        nc.vector.reciprocal(dinv[:, :], dnm[:, :])
        losscol = const.tile([128, NB], F32, tag="losscol")
        nc.vector.tensor_tensor(losscol[:, :], diff[:, :], dinv[:, :],
                                op=mybir.AluOpType.mult)

        lsum = const.tile([128, 1], F32, tag="lsum")
        nc.vector.tensor_reduce(lsum[:, :], losscol[:, :],
                                mybir.AxisListType.X, mybir.AluOpType.add)
        psf = psk_pool.tile([128, 512], F32)
        nc.tensor.matmul(psf[0:1, 0:1], lsum[:, :],
                         ones_col[:, :], start=True, stop=True)
        outsb = const.tile([1, 1], F32, tag="outsb")
        nc.scalar.copy(outsb[0:1, 0:1], psf[0:1, 0:1])
        nc.sync.dma_start(out_d[:, :], outsb[0:1, 0:1])

    nc.compile()
    return nc


# ---------------------------------------------------------------------------
# host-side marshalling
# ---------------------------------------------------------------------------

def make_inputs(q, k, y, cfg: Cfg):
    """Build the per-core input maps (pure layout/replication marshalling)."""
    N, D, KP, TW = cfg.N, cfg.D, cfg.KP, cfg.TW
    NL, NB, NS, KC, SW = cfg.NL, cfg.NB, cfg.NS, cfg.KC, cfg.SW
    q = np.asarray(q, dtype=np.float32)
    k = np.asarray(k, dtype=np.float32)
    y = np.asarray(y)

    qT = np.ascontiguousarray(q.T).reshape(KC, 128, N).astype(ml_dtypes.bfloat16)
    ybc = np.broadcast_to(y.astype(np.float16)[None, :], (128, N)).copy()
    colid = np.broadcast_to(np.arange(SW, dtype=np.float16)[None, :], (128, SW)).copy()
    # kmask[p, c] = 1 where c // KP == p (generalized-diagonal extractor)
    KW = KP * 128
    kmask = (np.arange(KW)[None, :] // KP == np.arange(128)[:, None]).astype(np.float16)

    in_maps = []
    p = np.arange(128)
    for r in range(NCORES):
        rows = slice(r * NL, (r + 1) * NL)
        qTl = np.ascontiguousarray(q[rows].T).reshape(KC, 128, NL).astype(ml_dtypes.bfloat16)
        # kTl[c, d', i*KP+kk] = k[r*NL+i, kk, c*128+d']
        kloc = k[rows].reshape(NL * KP, D)             # [(i,kk), d]
        kTl = np.ascontiguousarray(kloc.T).reshape(KC, 128, NL * KP).astype(ml_dtypes.bfloat16)
        yrow = np.ascontiguousarray(y[rows].astype(np.float32).reshape(NB, 128).T)
        # strip[p, b*NS+s] = within-strip diag col for partition p, or -1
        strip = np.full((128, NB * NS), -1.0, dtype=np.float32)
        for b in range(NB):
            grow = r * NL + b * 128 + p          # global rows of block b
            for s in range(NS):
                col = grow - s * TW              # within-tile diag col
                coff = b * 128 if SW == 128 else 0
                scol = col - coff                # within-strip col
                m = (scol >= 0) & (scol < SW) & (col >= 0) & (col < TW)
                strip[m, b * NS + s] = scol[m]
        in_maps.append({
            "qT": qT, "qTl": qTl, "kTl": kTl,
            "ybc": ybc, "yrow": yrow, "colid": colid, "strip": strip,
            "kmask": kmask,
        })
    return in_maps


_CACHE = {}


def _get_nc(cfg_key):
    if cfg_key not in _CACHE:
        cfg = Cfg()
        _CACHE[cfg_key] = (cfg, build_bass(cfg))
    return _CACHE[cfg_key]


def kernel(q, k, y, trace=False):
    cfg, nc = _get_nc("full")
    in_maps = make_inputs(q, k, y, cfg)
    res = run_bass_kernel_spmd(nc, in_maps, core_ids=list(range(NCORES)),
                               trace=trace)
    total = np.sum([res.results[r]["out"][0, 0] for r in range(NCORES)],
                   dtype=np.float64)
    out = np.asarray(total / cfg.N, dtype=np.float32)
    if trace:
        kernel.last_results = res
    return out
